# revision 5
# baseline (speedup 1.0000x reference)
"""Causal attention (flattened-head GQA variant) for TRN2, 8 NeuronCores.

Problem structure exploited:
  - K/V are group-projections tiled 4x along the head dim, and the score
    contraction runs over the full flattened 1024 dim.  Algebraically:
        att = Q @ tile(Kg,4)^T = (sum of Q's four 256-col blocks) @ Kg^T
        out = att_sm @ tile(Vg,4) = tile(att_sm @ Vg, 4)
    so the device only computes with 256-wide Qsum/Kg/Vg.
  - Softmax needs no max-subtraction here (logits bounded ~60; exp fits fp32
    comfortably), so scores are computed directly in the transposed layout
    U^T[s,t] = exp(Kg @ Qsum^T) and fed straight into the AV matmul as the
    stationary operand -- no on-device transposes at all.
  - Row sums come from a ones-column appended to Vg (PSUM col 256).
  - Block-causal skipping: s-tiles entirely above the diagonal are never
    computed; diagonal 128x256 blocks are masked with precomputed 0/1 tiles.

Schedule (v2): DMA queue ordered so each consumer is fed just in time --
wq, x0 (split in d-halves so projection starts at ~3.4us), wk, msk, x1,
bqk, wv, bvb, then x pair-chunks.  Weights go as single transfers with
512B contiguous runs (no sub-512B descriptor penalty).  A PE warm-up on a
zeroed 64-col tile covers the DMA lead-in and finishes the clock ramp
(pstate) before real work arrives.  Output tiles are written in fp16 and
DMAed in pairs to halve out-DMA time and shorten the final drain chain.

Precision: QK path in fp16 (11-bit mantissa, full PE rate, half DMA), scores
accumulated in fp32 PSUM, exp/AV path in bf16 (needs bf16's exponent range:
unnormalized exp values reach ~1e26).  fp16 output store adds <1e-3 rel.
End-to-end absmax rel error vs the fp32 reference ~8e-3.

Sharding: data-parallel over batch B=8, one batch per core, no collectives.
"""

import os
import numpy as np
import ml_dtypes
from contextlib import ExitStack

import concourse.tile as tile
from concourse import bacc, mybir
from concourse.bass_utils import run_bass_kernel_spmd

B, T, D = 8, 2048, 1024
C = 256          # group width (N_QUERY_GROUPS * HEAD_SIZE)
P = 128
ND = D // P      # 8 contraction tiles for projections
NS = T // P      # 16 s-tiles
JB = 256         # t-chunk width
NJB = T // JB    # 8
NCORES = 8

F32 = mybir.dt.float32
FP16 = mybir.dt.float16
BF16 = mybir.dt.bfloat16

N_WARM = 53      # warm-up matmuls of 64 rows each (fills DMA lead-in + ramp)


def _build():
    nc = bacc.Bacc("TRN2", target_bir_lowering=False, debug=False)
    xT = nc.dram_tensor("xT", [D, T], FP16, kind="ExternalInput").ap()
    wq = nc.dram_tensor("wq", [D, C], FP16, kind="ExternalInput").ap()
    wk = nc.dram_tensor("wk", [D, C], FP16, kind="ExternalInput").ap()
    wv = nc.dram_tensor("wv", [D, C], FP16, kind="ExternalInput").ap()
    bqk = nc.dram_tensor("bqk", [P, 4], F32, kind="ExternalInput").ap()
    bvb = nc.dram_tensor("bvb", [P, C], FP16, kind="ExternalInput").ap()
    msk = nc.dram_tensor("msk", [P, 2, JB], mybir.dt.float8e4, kind="ExternalInput").ap()
    o = nc.dram_tensor("o", [T, C], FP16, kind="ExternalOutput").ap()

    with tile.TileContext(nc) as tc, ExitStack() as ctx:
        cst = ctx.enter_context(tc.tile_pool(name="cst", bufs=1))
        big = ctx.enter_context(tc.tile_pool(name="big", bufs=1))
        up = ctx.enter_context(tc.tile_pool(name="up", bufs=3))
        outp = ctx.enter_context(tc.tile_pool(name="outp", bufs=3))
        pp = ctx.enter_context(tc.tile_pool(name="pp", bufs=2, space="PSUM"))
        pst = ctx.enter_context(tc.tile_pool(name="pst", bufs=4, space="PSUM"))
        pav = ctx.enter_context(tc.tile_pool(name="pav", bufs=2, space="PSUM"))

        bqk_t = cst.tile([P, 4], F32, tag="bqk")
        bvb_t = cst.tile([P, C], FP16, tag="bvb")
        msk_t = cst.tile([P, 2, JB], mybir.dt.float8e4, tag="msk")

        wr = {}
        for _n in ("q", "k", "v"):
            wr[_n] = cst.tile([P, ND, C], FP16, tag=f"w{_n}", name=f"wr_{_n}")

        # PE warm-up on a zeroed 64-col scratch: occupies the PE during the
        # DMA lead-in and completes the clock ramp (pstate) so real matmuls
        # run at full rate.  Tiny memset so the PE starts almost immediately.
        wrm = cst.tile([P, P], FP16, tag="wrm")
        nc.vector.memset(wrm[:], 0.0)
        for wi in range(N_WARM):
            ps_w = pp.tile([P, 2, JB], F32, tag="pp", name=f"warm_{wi}")
            nc.tensor.matmul(ps_w[:, 0, :64], wrm[:], wrm[:, :64],
                             start=True, stop=True)

        xtr = big.tile([P, ND, T], FP16, tag="xtr")
        qkT = {"q": big.tile([P, 2, T], FP16, tag="qsT", name="qsT"),
               "k": big.tile([P, 2, T], FP16, tag="ksT", name="ksT")}
        vg = big.tile([P, NS, C + 1], BF16, tag="vg")
        nc.vector.memset(vg[:, :, C:C + 1], 8.0)

        uts = {}

        def do_st(J):
            # scores^T -> exp for t-block J
            jt = slice(J * JB, (J + 1) * JB)
            ut = up.tile([P, NS, JB], BF16, tag="ut", name=f"ut_{J}")
            uts[J] = ut
            for sp in range(J + 1):
                si0 = 2 * sp
                ps_t = pst.tile([P, 2 * JB], F32, tag="pst",
                                name=f"pst_{J}_{sp}")
                if sp < J:
                    for h in range(2):
                        si = si0 + h
                        for ct in range(2):
                            nc.tensor.matmul(
                                ps_t[:, h * JB:(h + 1) * JB],
                                qkT["k"][:, ct, si * P:(si + 1) * P],
                                qkT["q"][:, ct, jt],
                                start=(ct == 0), stop=(ct == 1),
                            )
                    nc.scalar.activation(ut[:, si0:si0 + 2, :], ps_t[:],
                                         mybir.ActivationFunctionType.Exp)
                else:
                    # diagonal pair: si0 needs all 256 t-cols; si0+1 only
                    # its second 128 (AV q=0 never reads si0+1) -> N=128
                    for ct in range(2):
                        nc.tensor.matmul(
                            ps_t[:, 0:JB],
                            qkT["k"][:, ct, si0 * P:(si0 + 1) * P],
                            qkT["q"][:, ct, jt],
                            start=(ct == 0), stop=(ct == 1),
                        )
                    for ct in range(2):
                        nc.tensor.matmul(
                            ps_t[:, JB:JB + P],
                            qkT["k"][:, ct, (si0 + 1) * P:(si0 + 2) * P],
                            qkT["q"][:, ct, J * JB + P:(J + 1) * JB],
                            start=(ct == 0), stop=(ct == 1),
                        )
                    nc.scalar.activation(ut[:, si0, :], ps_t[:, 0:JB],
                                         mybir.ActivationFunctionType.Exp)
                    nc.scalar.activation(ut[:, si0 + 1, P:JB],
                                         ps_t[:, JB:JB + P],
                                         mybir.ActivationFunctionType.Exp)
                    nc.vector.tensor_tensor(ut[:, si0, :], ut[:, si0, :],
                                            msk_t[:, 0, :],
                                            mybir.AluOpType.mult)
                    nc.vector.tensor_tensor(ut[:, si0 + 1, P:JB],
                                            ut[:, si0 + 1, P:JB],
                                            msk_t[:, 1, P:JB],
                                            mybir.AluOpType.mult)

        def do_v(tb):
            # V projection for chunk tb's two s-tiles
            for si in (2 * tb, 2 * tb + 1):
                pv = pav.tile([P, C + 1], F32, tag="pav", name=f"pv_{si}")[:, :JB]
                for d in range(ND):
                    nc.tensor.matmul(
                        pv,
                        xtr[:, d, si * P:(si + 1) * P],
                        wr["v"][:, d, :],
                        start=(d == 0), stop=(d == ND - 1),
                    )
                nc.vector.tensor_tensor(vg[:, si, :C], pv, bvb_t[:],
                                        mybir.AluOpType.add)

        obt = {}

        def do_av(J):
            # AV for chunk J's two 128-row t-tiles; out pair DMAed together
            ut = uts[J]
            ob = outp.tile([P, 2, C], FP16, tag="ob", name=f"ob_{J}")
            obt[J] = ob
            for q in range(2):
                tci = 2 * J + q
                pa = pav.tile([P, C + 1], F32, tag="pav", name=f"pav_{J}_{q}")
                for si in range(tci + 1):
                    nc.tensor.matmul(
                        pa[:],
                        ut[:, si, q * P:(q + 1) * P],
                        vg[:, si, :],
                        start=(si == 0), stop=(si == tci),
                    )
                recip = outp.tile([P, 1], F32, tag="recip")
                nc.vector.reciprocal(recip[:], pa[:, C:C + 1])
                nc.vector.tensor_scalar_mul(ob[:, q, :], pa[:, :C], recip[:])
            nc.sync.dma_start(
                o[2 * J * P:(2 * J + 2) * P, :].rearrange(
                    "(two p) c -> p two c", p=P),
                ob[:])

        proj_ps = {}

        def do_proj(psl, mats=("q", "k"), dsl=slice(0, ND)):
            # Q/K projection for t-columns psl, contraction d-tiles dsl.
            # PSUM tile is one full bank [128, 2, 256(pw<=512/2)] holding both
            # ct halves; carried in proj_ps across a d-split (chunk 0).
            pw = psl.stop - psl.start
            d0, d1 = dsl.start, dsl.stop
            for mat in mats:
                mi = 0 if mat == "q" else 1
                for ct in range(2):
                    key = (mat, ct, psl.start)
                    if key not in proj_ps:
                        proj_ps[key] = pp.tile(
                            [P, 2 * JB], F32, tag="pp",
                            name=f"pp_{mat}{ct}_{psl.start}")
                    ps_p = proj_ps[key][:, :pw]
                    for d in range(d0, d1):
                        nc.tensor.matmul(
                            ps_p,
                            wr[mat][:, d, ct * P:(ct + 1) * P],
                            xtr[:, d, psl],
                            start=(d == 0), stop=(d == ND - 1),
                        )
                    if d1 == ND:
                        nc.vector.tensor_scalar_add(
                            qkT[mat][:, ct, psl],
                            ps_p,
                            bqk_t[:, 2 * mi + ct: 2 * mi + ct + 1],
                        )

        # ---- DMA queue: ordered for just-in-time consumer feed ----
        wq_src = wq.rearrange("(o p) c -> p o c", p=P)
        wk_src = wk.rearrange("(o p) c -> p o c", p=P)
        wv_src = wv.rearrange("(o p) c -> p o c", p=P)
        ts0 = slice(0, JB)
        xsrc0 = xT[:, ts0].rearrange("(o p) t -> p o t", p=P)
        nc.sync.dma_start(wr["q"][:], wq_src)
        nc.sync.dma_start(xtr[:, 0:4, ts0], xsrc0[:, 0:4, :])
        nc.sync.dma_start(xtr[:, 4:8, ts0], xsrc0[:, 4:8, :])
        nc.sync.dma_start(wr["k"][:], wk_src)
        nc.sync.dma_start(msk_t[:], msk)
        ts1 = slice(JB, 2 * JB)
        nc.sync.dma_start(xtr[:, :, ts1],
                          xT[:, ts1].rearrange("(o p) t -> p o t", p=P))
        nc.sync.dma_start(bqk_t[:], bqk)
        nc.sync.dma_start(wr["v"][:], wv_src)
        nc.sync.dma_start(bvb_t[:], bvb)
        for pb in (1, 2, 3):
            pts = slice(2 * pb * JB, (2 * pb + 2) * JB)
            nc.sync.dma_start(xtr[:, :, pts],
                              xT[:, pts].rearrange("(o p) t -> p o t", p=P))

        # ---- PE order ----
        # chunk 0 with d-split so the first projection starts on x0a
        do_proj(ts0, mats=("q",), dsl=slice(0, 4))
        do_proj(ts0, mats=("q",), dsl=slice(4, 8))
        do_proj(ts0, mats=("k",), dsl=slice(0, 4))
        do_proj(ts0, mats=("k",), dsl=slice(4, 8))
        do_st(0)
        do_proj(ts1)
        do_st(1)
        do_v(0)
        do_av(0)
        do_v(1)
        do_av(1)
        for pb in (1, 2, 3):
            tb0 = 2 * pb
            pts = slice(tb0 * JB, (tb0 + 2) * JB)
            do_proj(pts)
            for tb in (tb0, tb0 + 1):
                do_st(tb)
                do_v(tb)
                do_av(tb)

    nc.compile()
    return nc


_CACHE = {}
LAST_EXEC_TIME_NS = None


def _get_nc():
    if "nc" not in _CACHE:
        _CACHE["nc"] = _build()
    return _CACHE["nc"]


def kernel(x, Wq, bq, Wk, bk, Wv, bv):
    x = np.asarray(x, dtype=np.float32)
    Wq = np.asarray(Wq, dtype=np.float32)
    bq = np.asarray(bq, dtype=np.float32)
    Wk = np.asarray(Wk, dtype=np.float32)
    bk = np.asarray(bk, dtype=np.float32)
    Wv = np.asarray(Wv, dtype=np.float32)
    bv = np.asarray(bv, dtype=np.float32)

    # Fold the 4x head-tiling into the weights: contraction with tile(Kg,4)
    # equals contraction of block-summed Q with Kg.
    wq_s = Wq.reshape(D, 4, C).sum(axis=1, dtype=np.float64).astype(np.float32)
    bq_s = bq.reshape(4, C).sum(axis=0, dtype=np.float64).astype(np.float32)

    bqk = np.stack([bq_s[:P], bq_s[P:], bk[:P], bk[P:]], axis=1).astype(np.float32)
    bvb = np.broadcast_to(bv, (P, C)).astype(np.float32)

    # Diagonal-block causal masks: keep t >= s  <=>  j >= 128*m + p.
    jj = np.arange(JB)[None, None, :]
    pp_ = np.arange(P)[:, None, None]
    mm = np.arange(2)[None, :, None]
    msk = (jj >= P * mm + pp_).astype(ml_dtypes.float8_e4m3)

    shared = {
        "wq": np.ascontiguousarray(wq_s.astype(np.float16)),
        "wk": np.ascontiguousarray(Wk.astype(np.float16)),
        "wv": np.ascontiguousarray(Wv.astype(np.float16)),
        "bqk": np.ascontiguousarray(bqk), "bvb": np.ascontiguousarray(bvb.astype(np.float16)),
        "msk": np.ascontiguousarray(msk),
    }
    in_maps = []
    for b in range(B):
        m = dict(shared)
        m["xT"] = np.ascontiguousarray(x[b].T.astype(np.float16))
        in_maps.append(m)

    nc = _get_nc()
    try:
        res = run_bass_kernel_spmd(nc, in_maps, core_ids=list(range(NCORES)))
    except ModuleNotFoundError:
        # BASS_TRACE=1 requests NTFF profiling, but this container type has
        # no axon NTFF hook (antenv.axon_hooks absent) -- rerun untraced.
        os.environ["BASS_NEVER_TRACE"] = "1"
        res = run_bass_kernel_spmd(nc, in_maps, core_ids=list(range(NCORES)))
    global LAST_EXEC_TIME_NS
    LAST_EXEC_TIME_NS = res.exec_time_ns
    if res.exec_time_ns is not None:
        print(f"HW exec time: {res.exec_time_ns} ns")

    out = np.empty((1, B, T, 4 * C), dtype=np.float32)
    for b in range(B):
        ob = res.results[b]["o"].astype(np.float32)
        out[0, b] = np.tile(ob, (1, 4))
    return out


# revision 6
# speedup vs baseline: 1.0306x; 1.0306x over previous
"""Causal attention (flattened-head GQA variant) for TRN2, 8 NeuronCores.

Problem structure exploited:
  - K/V are group-projections tiled 4x along the head dim, and the score
    contraction runs over the full flattened 1024 dim.  Algebraically:
        att = Q @ tile(Kg,4)^T = (sum of Q's four 256-col blocks) @ Kg^T
        out = att_sm @ tile(Vg,4) = tile(att_sm @ Vg, 4)
    so the device only computes with 256-wide Qsum/Kg/Vg.
  - Softmax needs no max-subtraction here (logits bounded ~60; exp fits fp32
    comfortably), so scores are computed directly in the transposed layout
    U^T[s,t] = exp(Kg @ Qsum^T) and fed straight into the AV matmul as the
    stationary operand -- no on-device transposes at all.
  - Row sums come from a ones-column appended to Vg (PSUM col 256).
  - Block-causal skipping: s-tiles entirely above the diagonal are never
    computed; diagonal 128x256 blocks are masked with precomputed 0/1 tiles.

Schedule (v2): DMA queue ordered so each consumer is fed just in time --
wq, x0 (split in d-halves so projection starts at ~3.4us), wk, msk, x1,
bqk, wv, bvb, then x pair-chunks.  Weights go as single transfers with
512B contiguous runs (no sub-512B descriptor penalty).  A PE warm-up on a
zeroed 64-col tile covers the DMA lead-in and finishes the clock ramp
(pstate) before real work arrives.  Output tiles are written in fp16 and
DMAed in pairs to halve out-DMA time and shorten the final drain chain.

Precision: QK path in fp16 (11-bit mantissa, full PE rate, half DMA), scores
accumulated in fp32 PSUM, exp/AV path in bf16 (needs bf16's exponent range:
unnormalized exp values reach ~1e26).  fp16 output store adds <1e-3 rel.
End-to-end absmax rel error vs the fp32 reference ~8e-3.

Sharding: data-parallel over batch B=8, one batch per core, no collectives.
"""

import os
import numpy as np
import ml_dtypes
from contextlib import ExitStack

import concourse.tile as tile
from concourse import bacc, mybir
from concourse.bass_utils import run_bass_kernel_spmd

B, T, D = 8, 2048, 1024
C = 256          # group width (N_QUERY_GROUPS * HEAD_SIZE)
P = 128
ND = D // P      # 8 contraction tiles for projections
NS = T // P      # 16 s-tiles
JB = 256         # t-chunk width
NJB = T // JB    # 8
NCORES = 8

F32 = mybir.dt.float32
FP16 = mybir.dt.float16
BF16 = mybir.dt.bfloat16

N_WARM = 23      # warm-up matmuls of 256 rows each (fills DMA lead-in + ramp)


def _build():
    nc = bacc.Bacc("TRN2", target_bir_lowering=False, debug=False)
    xT = nc.dram_tensor("xT", [D, T], FP16, kind="ExternalInput").ap()
    wq = nc.dram_tensor("wq", [D, C], FP16, kind="ExternalInput").ap()
    wk = nc.dram_tensor("wk", [D, C], FP16, kind="ExternalInput").ap()
    wv = nc.dram_tensor("wv", [D, C], FP16, kind="ExternalInput").ap()
    bqk = nc.dram_tensor("bqk", [P, 4], F32, kind="ExternalInput").ap()
    bvb = nc.dram_tensor("bvb", [P, C], FP16, kind="ExternalInput").ap()
    msk = nc.dram_tensor("msk", [P, 2, JB], mybir.dt.float8e4, kind="ExternalInput").ap()
    o = nc.dram_tensor("o", [T, C], FP16, kind="ExternalOutput").ap()

    with tile.TileContext(nc) as tc, ExitStack() as ctx:
        cst = ctx.enter_context(tc.tile_pool(name="cst", bufs=1))
        big = ctx.enter_context(tc.tile_pool(name="big", bufs=1))
        up = ctx.enter_context(tc.tile_pool(name="up", bufs=3))
        outp = ctx.enter_context(tc.tile_pool(name="outp", bufs=3))
        pp = ctx.enter_context(tc.tile_pool(name="pp", bufs=2, space="PSUM"))
        pst = ctx.enter_context(tc.tile_pool(name="pst", bufs=4, space="PSUM"))
        pav = ctx.enter_context(tc.tile_pool(name="pav", bufs=2, space="PSUM"))

        bqk_t = cst.tile([P, 4], F32, tag="bqk")
        bvb_t = cst.tile([P, C], FP16, tag="bvb")
        msk_t = cst.tile([P, 2, JB], mybir.dt.float8e4, tag="msk")

        wr = {}
        for _n in ("q", "k", "v"):
            wr[_n] = cst.tile([P, ND, C], FP16, tag=f"w{_n}", name=f"wr_{_n}")

        # PE warm-up on a zeroed 64-col scratch: occupies the PE during the
        # DMA lead-in and completes the clock ramp (pstate) so real matmuls
        # run at full rate.  Tiny memset so the PE starts almost immediately.
        wrm = cst.tile([P, 2 * P], FP16, tag="wrm")
        nc.vector.memset(wrm[:], 0.0)
        for wi in range(N_WARM):
            ps_w = pp.tile([P, 2 * JB], F32, tag="pp", name=f"warm_{wi}")
            nc.tensor.matmul(ps_w[:, :2 * P], wrm[:, :P], wrm[:],
                             start=True, stop=True)

        xtr = big.tile([P, ND, T], FP16, tag="xtr")
        qkT = {"q": big.tile([P, 2, T], FP16, tag="qsT", name="qsT"),
               "k": big.tile([P, 2, T], FP16, tag="ksT", name="ksT")}
        vg = big.tile([P, NS, C + 1], BF16, tag="vg")
        nc.vector.memset(vg[:, :, C:C + 1], 8.0)

        uts = {}

        def do_st(J):
            # scores^T -> exp for t-block J
            jt = slice(J * JB, (J + 1) * JB)
            ut = up.tile([P, NS, JB], BF16, tag="ut", name=f"ut_{J}")
            uts[J] = ut
            for sp in range(J + 1):
                si0 = 2 * sp
                ps_t = pst.tile([P, 2 * JB], F32, tag="pst",
                                name=f"pst_{J}_{sp}")
                if sp < J:
                    for h in range(2):
                        si = si0 + h
                        for ct in range(2):
                            nc.tensor.matmul(
                                ps_t[:, h * JB:(h + 1) * JB],
                                qkT["k"][:, ct, si * P:(si + 1) * P],
                                qkT["q"][:, ct, jt],
                                start=(ct == 0), stop=(ct == 1),
                            )
                    nc.scalar.activation(ut[:, si0:si0 + 2, :], ps_t[:],
                                         mybir.ActivationFunctionType.Exp)
                else:
                    # diagonal pair: si0 needs all 256 t-cols; si0+1 only
                    # its second 128 (AV q=0 never reads si0+1) -> N=128
                    for ct in range(2):
                        nc.tensor.matmul(
                            ps_t[:, 0:JB],
                            qkT["k"][:, ct, si0 * P:(si0 + 1) * P],
                            qkT["q"][:, ct, jt],
                            start=(ct == 0), stop=(ct == 1),
                        )
                    for ct in range(2):
                        nc.tensor.matmul(
                            ps_t[:, JB:JB + P],
                            qkT["k"][:, ct, (si0 + 1) * P:(si0 + 2) * P],
                            qkT["q"][:, ct, J * JB + P:(J + 1) * JB],
                            start=(ct == 0), stop=(ct == 1),
                        )
                    nc.scalar.activation(ut[:, si0, :], ps_t[:, 0:JB],
                                         mybir.ActivationFunctionType.Exp)
                    nc.scalar.activation(ut[:, si0 + 1, P:JB],
                                         ps_t[:, JB:JB + P],
                                         mybir.ActivationFunctionType.Exp)
                    nc.vector.tensor_tensor(ut[:, si0, :], ut[:, si0, :],
                                            msk_t[:, 0, :],
                                            mybir.AluOpType.mult)
                    nc.vector.tensor_tensor(ut[:, si0 + 1, P:JB],
                                            ut[:, si0 + 1, P:JB],
                                            msk_t[:, 1, P:JB],
                                            mybir.AluOpType.mult)

        v_ps = {}

        def do_v(tb, dsl=slice(0, ND)):
            # V projection for chunk tb's two s-tiles
            d0, d1 = dsl.start, dsl.stop
            for si in (2 * tb, 2 * tb + 1):
                if si not in v_ps:
                    v_ps[si] = pav.tile([P, C + 1], F32, tag="pav",
                                        name=f"pv_{si}")
                pv = v_ps[si][:, :JB]
                for d in range(d0, d1):
                    nc.tensor.matmul(
                        pv,
                        xtr[:, d, si * P:(si + 1) * P],
                        wr["v"][:, d, :],
                        start=(d == 0), stop=(d == ND - 1),
                    )
                if d1 == ND:
                    nc.vector.tensor_tensor(vg[:, si, :C], pv, bvb_t[:],
                                            mybir.AluOpType.add)

        obt = {}

        def do_av(J):
            # AV for chunk J's two 128-row t-tiles; out pair DMAed together
            ut = uts[J]
            ob = outp.tile([P, 2, C], FP16, tag="ob", name=f"ob_{J}")
            obt[J] = ob
            for q in range(2):
                tci = 2 * J + q
                pa = pav.tile([P, C + 1], F32, tag="pav", name=f"pav_{J}_{q}")
                for si in range(tci + 1):
                    nc.tensor.matmul(
                        pa[:],
                        ut[:, si, q * P:(q + 1) * P],
                        vg[:, si, :],
                        start=(si == 0), stop=(si == tci),
                    )
                recip = outp.tile([P, 1], F32, tag="recip")
                nc.vector.reciprocal(recip[:], pa[:, C:C + 1])
                nc.vector.tensor_scalar_mul(ob[:, q, :], pa[:, :C], recip[:])
            nc.sync.dma_start(
                o[2 * J * P:(2 * J + 2) * P, :].rearrange(
                    "(two p) c -> p two c", p=P),
                ob[:])

        proj_ps = {}

        def do_proj(psl, mats=("q", "k"), dsl=slice(0, ND)):
            # Q/K projection for t-columns psl, contraction d-tiles dsl.
            # PSUM tile is one full bank [128, 2, 256(pw<=512/2)] holding both
            # ct halves; carried in proj_ps across a d-split (chunk 0).
            pw = psl.stop - psl.start
            d0, d1 = dsl.start, dsl.stop
            for mat in mats:
                mi = 0 if mat == "q" else 1
                for ct in range(2):
                    key = (mat, ct, psl.start)
                    if key not in proj_ps:
                        proj_ps[key] = pp.tile(
                            [P, 2 * JB], F32, tag="pp",
                            name=f"pp_{mat}{ct}_{psl.start}")
                    ps_p = proj_ps[key][:, :pw]
                    for d in range(d0, d1):
                        nc.tensor.matmul(
                            ps_p,
                            wr[mat][:, d, ct * P:(ct + 1) * P],
                            xtr[:, d, psl],
                            start=(d == 0), stop=(d == ND - 1),
                        )
                    if d1 == ND:
                        nc.vector.tensor_scalar_add(
                            qkT[mat][:, ct, psl],
                            ps_p,
                            bqk_t[:, 2 * mi + ct: 2 * mi + ct + 1],
                        )

        # ---- DMA queue: ordered for just-in-time consumer feed.  All of
        # wq/x0/wk/wv go before x1 (each weight piece unlocks more pre-x1 PE
        # work than its transfer costs); halves let consumers start earlier.
        wq_src = wq.rearrange("(o p) c -> p o c", p=P)
        wk_src = wk.rearrange("(o p) c -> p o c", p=P)
        wv_src = wv.rearrange("(o p) c -> p o c", p=P)
        ts0 = slice(0, JB)
        ts1 = slice(JB, 2 * JB)
        xsrc0 = xT[:, ts0].rearrange("(o p) t -> p o t", p=P)
        nc.sync.dma_start(wr["q"][:, 0:4], wq_src[:, 0:4])
        nc.sync.dma_start(xtr[:, 0:4, ts0], xsrc0[:, 0:4, :])
        nc.sync.dma_start(bqk_t[:], bqk)
        nc.sync.dma_start(wr["q"][:, 4:8], wq_src[:, 4:8])
        nc.sync.dma_start(xtr[:, 4:8, ts0], xsrc0[:, 4:8, :])
        nc.sync.dma_start(wr["k"][:, 0:4], wk_src[:, 0:4])
        nc.sync.dma_start(wr["k"][:, 4:8], wk_src[:, 4:8])
        nc.sync.dma_start(wr["v"][:, 0:4], wv_src[:, 0:4])
        nc.sync.dma_start(wr["v"][:, 4:8], wv_src[:, 4:8])
        nc.sync.dma_start(bvb_t[:], bvb)
        nc.sync.dma_start(msk_t[:], msk)
        nc.sync.dma_start(xtr[:, :, ts1],
                          xT[:, ts1].rearrange("(o p) t -> p o t", p=P))
        for pb in (1, 2, 3):
            pts = slice(2 * pb * JB, (2 * pb + 2) * JB)
            nc.sync.dma_start(xtr[:, :, pts],
                              xT[:, pts].rearrange("(o p) t -> p o t", p=P))

        # ---- PE order: everything x0-dependent runs before the x1 gate ----
        do_proj(ts0, mats=("q",), dsl=slice(0, 4))
        do_proj(ts0, mats=("q",), dsl=slice(4, 8))
        do_proj(ts0, mats=("k",), dsl=slice(0, 4))
        do_proj(ts0, mats=("k",), dsl=slice(4, 8))
        do_st(0)
        do_v(0, dsl=slice(0, 4))
        do_v(0, dsl=slice(4, 8))
        do_proj(ts1)
        do_av(0)
        do_v(1)
        do_st(1)
        do_av(1)
        for pb in (1, 2, 3):
            tb0 = 2 * pb
            pts = slice(tb0 * JB, (tb0 + 2) * JB)
            do_proj(pts)
            for tb in (tb0, tb0 + 1):
                do_st(tb)
                do_v(tb)
                do_av(tb)

    nc.compile()
    return nc


_CACHE = {}
LAST_EXEC_TIME_NS = None


def _get_nc():
    if "nc" not in _CACHE:
        _CACHE["nc"] = _build()
    return _CACHE["nc"]


def kernel(x, Wq, bq, Wk, bk, Wv, bv):
    x = np.asarray(x, dtype=np.float32)
    Wq = np.asarray(Wq, dtype=np.float32)
    bq = np.asarray(bq, dtype=np.float32)
    Wk = np.asarray(Wk, dtype=np.float32)
    bk = np.asarray(bk, dtype=np.float32)
    Wv = np.asarray(Wv, dtype=np.float32)
    bv = np.asarray(bv, dtype=np.float32)

    # Fold the 4x head-tiling into the weights: contraction with tile(Kg,4)
    # equals contraction of block-summed Q with Kg.
    wq_s = Wq.reshape(D, 4, C).sum(axis=1, dtype=np.float64).astype(np.float32)
    bq_s = bq.reshape(4, C).sum(axis=0, dtype=np.float64).astype(np.float32)

    bqk = np.stack([bq_s[:P], bq_s[P:], bk[:P], bk[P:]], axis=1).astype(np.float32)
    bvb = np.broadcast_to(bv, (P, C)).astype(np.float32)

    # Diagonal-block causal masks: keep t >= s  <=>  j >= 128*m + p.
    jj = np.arange(JB)[None, None, :]
    pp_ = np.arange(P)[:, None, None]
    mm = np.arange(2)[None, :, None]
    msk = (jj >= P * mm + pp_).astype(ml_dtypes.float8_e4m3)

    shared = {
        "wq": np.ascontiguousarray(wq_s.astype(np.float16)),
        "wk": np.ascontiguousarray(Wk.astype(np.float16)),
        "wv": np.ascontiguousarray(Wv.astype(np.float16)),
        "bqk": np.ascontiguousarray(bqk), "bvb": np.ascontiguousarray(bvb.astype(np.float16)),
        "msk": np.ascontiguousarray(msk),
    }
    in_maps = []
    for b in range(B):
        m = dict(shared)
        m["xT"] = np.ascontiguousarray(x[b].T.astype(np.float16))
        in_maps.append(m)

    nc = _get_nc()
    try:
        res = run_bass_kernel_spmd(nc, in_maps, core_ids=list(range(NCORES)))
    except ModuleNotFoundError:
        # BASS_TRACE=1 requests NTFF profiling, but this container type has
        # no axon NTFF hook (antenv.axon_hooks absent) -- rerun untraced.
        os.environ["BASS_NEVER_TRACE"] = "1"
        res = run_bass_kernel_spmd(nc, in_maps, core_ids=list(range(NCORES)))
    global LAST_EXEC_TIME_NS
    LAST_EXEC_TIME_NS = res.exec_time_ns
    if res.exec_time_ns is not None:
        print(f"HW exec time: {res.exec_time_ns} ns")

    out = np.empty((1, B, T, 4 * C), dtype=np.float32)
    for b in range(B):
        ob = res.results[b]["o"].astype(np.float32)
        out[0, b] = np.tile(ob, (1, 4))
    return out
